# revision 7
# baseline (speedup 1.0000x reference)
"""Trainium2 Bass kernel for nn_Cross_modal_ContrastiveLoss6.

Math: the reference loss only depends on per-class means of the two
modalities (every row of the N x N distance matrix is determined by the
class pair), so the whole computation reduces to:

  1. raw per-class segment sums R[c,d], T[c,d]  (memory-bound: 64 MiB read)
  2. the three 128x128 class Gram matrices P1 = R R^T, P2 = T T^T, P3 = R T^T
  3. tiny 128x128 class-pair loss math with the class counts

Device strategy (8 cores, feature/d-sharded so no cross-core collective is
needed before the nonlinearity): core k takes columns [256k, 256k+256) of
both modal tensors, computes the full-N segment sums for its d-chunk via
one-hot matmuls on the PE (data as stationary operand -> sums come out
d-on-partitions, exactly the layout the Gram contraction needs), then the
three partial Grams over its d-chunk. Host sums the 8 partial Grams and
applies the count scaling + sqrt/relu/weighted-mean (0.1% of the FLOPs).
"""

import numpy as np

import concourse.bass as bass
import concourse.bacc as bacc
import concourse.mybir as mybir
from concourse.bass_utils import run_bass_kernel_spmd
from concourse.tile import TileContext

N = 4096
D = 2048
C = 128
MARGIN = 0.5
NCORES = 8
DCHUNK = D // NCORES          # 256 feature columns per core
P = 128                       # partitions / sample-block size
NB = N // P                   # 32 sample blocks
CHUNK_B = 8                   # sample blocks per DMA (1 MiB per modal chunk)
NCHUNK = NB // CHUNK_B        # 4 DMA chunks per modal
H = DCHUNK // P               # 2 column halves (stationary operand is <=128 wide)

F32 = mybir.dt.float32

_PROGRAM = None


def _build_program() -> bass.Bass:
    nc = bacc.Bacc()

    x1 = nc.declare_dram_parameter("x1", [N, DCHUNK], F32, isOutput=False)
    x2 = nc.declare_dram_parameter("x2", [N, DCHUNK], F32, isOutput=False)
    # consts[:, :C] = iota (iota[p, c] = c), consts[:, C:C+NB] = targets
    # laid out as tgt[p, b] = targets[b*128 + p]; one tensor -> one DMA -> a
    # single semaphore wait on the first consumer.
    consts_in = nc.declare_dram_parameter("consts", [P, C + NB], F32, isOutput=False)
    grams = nc.declare_dram_parameter("grams", [P, 3 * C], F32, isOutput=True)

    with TileContext(nc) as tc:
        with (
            tc.tile_pool(name="consts", bufs=1) as consts,
            tc.tile_pool(name="oh", bufs=1) as ohpool,
            tc.tile_pool(name="xin", bufs=3) as xpool,
            tc.tile_pool(name="outs", bufs=1) as opool,
            tc.tile_pool(name="acc", bufs=1, space="PSUM") as accpool,
        ):
            const_t = consts.tile([P, C + NB], F32)
            nc.sync.dma_start(out=const_t[:], in_=consts_in[:])

            # One-hot blocks: oh_t[p, b, c] = (targets[b*128+p] == c)
            oh_t = ohpool.tile([P, NB, C], F32)
            for b in range(NB):
                nc.vector.tensor_scalar(
                    oh_t[:, b, :],
                    const_t[:, 0:C],
                    const_t[:, C + b : C + b + 1],
                    None,
                    mybir.AluOpType.is_equal,
                )

            # Transposed segment-sum accumulators in PSUM:
            # acc[m][h][d, c] = sum_i X_m[i, h*128+d] * onehot[i, c]
            acc = [
                [
                    accpool.tile([P, C], F32, name=f"acc_{m}_{h}", tag=f"acc_{m}_{h}")
                    for h in range(H)
                ]
                for m in range(2)
            ]

            x1r = x1[:].rearrange("(j b p) d -> j p b d", b=CHUNK_B, p=P)
            x2r = x2[:].rearrange("(j b p) d -> j p b d", b=CHUNK_B, p=P)
            for j in range(NCHUNK):
                x1_t = xpool.tile([P, CHUNK_B, DCHUNK], F32, name="x1_t", tag="x1_t")
                x2_t = xpool.tile([P, CHUNK_B, DCHUNK], F32, name="x2_t", tag="x2_t")
                nc.sync.dma_start(out=x1_t[:], in_=x1r[j])
                nc.sync.dma_start(out=x2_t[:], in_=x2r[j])
                for bb in range(CHUNK_B):
                    b = j * CHUNK_B + bb
                    for m, xt in ((0, x1_t), (1, x2_t)):
                        for h in range(H):
                            nc.tensor.matmul(
                                acc[m][h][:],
                                xt[:, bb, h * P : (h + 1) * P],
                                oh_t[:, b, :],
                                start=(b == 0),
                                stop=(b == NB - 1),
                            )

            # Sums to SBUF: sums_t[d, (m, h, c)]
            sums_t = opool.tile([P, 2, H, C], F32)
            for m in range(2):
                for h in range(H):
                    nc.vector.tensor_copy(sums_t[:, m, h, :], acc[m][h][:])

            # Partial Grams over this core's d-chunk:
            # P1 = R R^T, P2 = T T^T, P3 = R T^T  (contraction over d = partitions)
            gram_ps = [
                accpool.tile([P, C], F32, name=f"gram_{i}", tag=f"gram_{i}")
                for i in range(3)
            ]
            for i, (ma, mb) in enumerate(((0, 0), (1, 1), (0, 1))):
                for h in range(H):
                    nc.tensor.matmul(
                        gram_ps[i][:],
                        sums_t[:, ma, h, :],
                        sums_t[:, mb, h, :],
                        start=(h == 0),
                        stop=(h == H - 1),
                    )

            out_t = opool.tile([P, 3 * C], F32)
            for i in range(3):
                nc.vector.tensor_copy(out_t[:, i * C : (i + 1) * C], gram_ps[i][:])
            nc.sync.dma_start(out=grams[:], in_=out_t[:])

    nc.compile()
    return nc


def _get_program() -> bass.Bass:
    global _PROGRAM
    if _PROGRAM is None:
        _PROGRAM = _build_program()
    return _PROGRAM


def _make_in_maps(modal1, modal2, targets):
    x1 = np.ascontiguousarray(modal1, dtype=np.float32)
    x2 = np.ascontiguousarray(modal2, dtype=np.float32)
    tgt_pb = targets.reshape(NB, P).T.astype(np.float32)  # [p, b] = targets[b*128+p]
    iota = np.tile(np.arange(C, dtype=np.float32), (P, 1))  # iota[p, c] = c
    consts = np.ascontiguousarray(np.concatenate([iota, tgt_pb], axis=1))
    in_maps = []
    for k in range(NCORES):
        sl = slice(k * DCHUNK, (k + 1) * DCHUNK)
        in_maps.append(
            {
                "x1": np.ascontiguousarray(x1[:, sl]),
                "x2": np.ascontiguousarray(x2[:, sl]),
                "consts": consts,
            }
        )
    return in_maps


def _finish_on_host(gram_list, targets):
    """Sum per-core partial Grams and do the 128x128 class-pair loss math."""
    P1 = np.zeros((C, C), np.float64)
    P2 = np.zeros((C, C), np.float64)
    P3 = np.zeros((C, C), np.float64)
    for g in gram_list:
        g = np.asarray(g, np.float64)
        P1 += g[:, 0 * C : 1 * C]
        P2 += g[:, 1 * C : 2 * C]
        P3 += g[:, 2 * C : 3 * C]

    n = np.bincount(targets, minlength=C).astype(np.float64)
    u = 1.0 / np.maximum(n, 1.0)

    S_RC = P1 + P3          # R (R+T)^T
    S_TC = P2 + P3.T        # T (R+T)^T
    S_CC = P1 + P2 + P3 + P3.T  # (R+T)(R+T)^T

    uu = np.outer(u, u)
    A1 = 0.5 * uu * S_RC    # meanR . ctr
    A2 = 0.5 * uu * S_TC    # meanT . ctr
    nR = u * u * np.diag(P1)
    nT = u * u * np.diag(P2)
    nCtr = 0.25 * u * u * np.diag(S_CC)

    W = np.outer(n, n)
    eye = np.eye(C)
    total = 0.0
    for A, nrm in ((A1, nR), (A2, nT)):
        sq = np.maximum(nrm[:, None] + nCtr[None, :] - 2.0 * A, 1e-12)
        d = np.sqrt(sq)
        dd = np.sqrt(d + 1e-10)
        term = eye * sq + (1.0 - eye) * np.maximum(MARGIN - dd, 0.0) ** 2
        total += (W * term).sum() / (float(N) * float(N))
    return np.asarray(total, dtype=np.float32)


def kernel(modal1_inputs, modal2_inputs, targets):
    nc = _get_program()
    in_maps = _make_in_maps(modal1_inputs, modal2_inputs, targets)
    res = run_bass_kernel_spmd(nc, in_maps, list(range(NCORES)))
    gram_list = [res.results[k]["grams"] for k in range(NCORES)]
    return _finish_on_host(gram_list, np.asarray(targets))


# revision 8
# speedup vs baseline: 1.1316x; 1.1316x over previous
"""Trainium2 Bass kernel for nn_Cross_modal_ContrastiveLoss6.

Math: the reference loss only depends on per-class means of the two
modalities (every entry of the N x N distance matrix is determined by the
class pair), so the whole computation reduces to:

  1. raw per-class segment sums R[c,d], T[c,d]  (memory-bound: 64 MiB read)
  2. the three 128x128 class Gram matrices P1 = R R^T, P2 = T T^T, P3 = R T^T
  3. tiny 128x128 class-pair loss math with the class counts

Device strategy (8 cores, feature/d-sharded so no cross-core collective is
needed): core k takes columns [256k, 256k+256) of both modal tensors and
computes the full-N segment sums for its d-chunk with one-hot matmuls on
the PE.  fp32 matmuls run at 1/8 the bf16 rate on trn2, so the host splits
the fp32 data into bf16 (hi, lo) pairs -- exact to ~2^-17 relative, same
total DMA bytes -- and the one-hot matrix is precomputed on the host in
bf16 (0/1 exact).  Everything is packed host-side into the exact SBUF
layout ([128 partitions, free]) so each DMA is a flat contiguous copy.
The device returns the raw hi/lo segment sums; the host recombines them,
forms the three Grams, and does the count scaling + sqrt/relu/weighted
mean (<0.1% of the FLOPs) in float64.
"""

import numpy as np
import ml_dtypes

import concourse.bacc as bacc
import concourse.bass as bass
import concourse.mybir as mybir
from concourse.bass_utils import run_bass_kernel_spmd
from concourse.tile import TileContext

N = 4096
D = 2048
C = 128
MARGIN = 0.5
NCORES = 8
DCHUNK = D // NCORES          # 256 feature columns per core
P = 128                       # partitions / sample-block size
NB = N // P                   # 32 sample blocks
CHUNK_B = 4                   # sample blocks per x-DMA (512 KiB)
NCHUNK = NB // CHUNK_B        # 8 DMA chunks per modal
OH_CHUNK_B = 8                # sample blocks per one-hot DMA (256 KiB)

F32 = mybir.dt.float32
BF16 = mybir.dt.bfloat16
NPBF16 = ml_dtypes.bfloat16

_PROGRAM = None


def _build_program() -> bass.Bass:
    nc = bacc.Bacc()

    # All inputs are packed host-side as [128 partitions, free] where
    # partition p of sample-block b is sample b*128+p.
    oh_in = nc.declare_dram_parameter("oh", [P, NB * C], BF16, isOutput=False)
    x1_in = nc.declare_dram_parameter("x1", [P, NB * 512], BF16, isOutput=False)
    x2_in = nc.declare_dram_parameter("x2", [P, NB * 512], BF16, isOutput=False)
    # sums[:, 0:512]  = R hi|lo partial sums   [class, 2*256]
    # sums[:, 512:1024] = T hi|lo partial sums
    sums_out = nc.declare_dram_parameter("sums", [P, 1024], F32, isOutput=True)

    with TileContext(nc) as tc:
        with (
            tc.tile_pool(name="data", bufs=1) as data,
            tc.tile_pool(name="acc", bufs=1, space="PSUM") as accpool,
        ):
            # Whole input stays resident (73 KiB/partition of 208).
            oh_t = data.tile([P, NB * C], BF16)
            x1_t = data.tile([P, NB * 512], BF16)
            x2_t = data.tile([P, NB * 512], BF16)

            psum_r = accpool.tile([P, 512], F32, name="psum_r", tag="psum_r")
            psum_t = accpool.tile([P, 512], F32, name="psum_t", tag="psum_t")

            # DMAs: interleave one-hot and x chunks so the PE can start as
            # soon as the first ~1.25 MiB has landed.  x1 chunks are issued
            # from the sync ring, x2 chunks from the scalar ring.
            for j in range(NCHUNK):
                if j % 2 == 0:
                    ohj = j // 2
                    nc.sync.dma_start(
                        out=oh_t[:, ohj * OH_CHUNK_B * C : (ohj + 1) * OH_CHUNK_B * C],
                        in_=oh_in[:, ohj * OH_CHUNK_B * C : (ohj + 1) * OH_CHUNK_B * C],
                    )
                sl = slice(j * CHUNK_B * 512, (j + 1) * CHUNK_B * 512)
                nc.sync.dma_start(out=x1_t[:, sl], in_=x1_in[:, sl])
                nc.scalar.dma_start(out=x2_t[:, sl], in_=x2_in[:, sl])

            # Segment sums: psum_r[c, n] = sum_b sum_p oh[p, b*C+c] * x1[p, b*512+n]
            # (hi sums land in cols 0:256, lo sums in 256:512; host adds them).
            for b in range(NB):
                ohs = oh_t[:, b * C : (b + 1) * C]
                nc.tensor.matmul(
                    psum_r[:],
                    ohs,
                    x1_t[:, b * 512 : (b + 1) * 512],
                    start=(b == 0),
                    stop=(b == NB - 1),
                )
                nc.tensor.matmul(
                    psum_t[:],
                    ohs,
                    x2_t[:, b * 512 : (b + 1) * 512],
                    start=(b == 0),
                    stop=(b == NB - 1),
                )

            out_t = data.tile([P, 1024], F32)
            nc.vector.tensor_copy(out_t[:, 0:512], psum_r[:])
            nc.vector.tensor_copy(out_t[:, 512:1024], psum_t[:])
            nc.sync.dma_start(out=sums_out[:], in_=out_t[:])

    nc.compile()
    return nc


def _get_program() -> bass.Bass:
    global _PROGRAM
    if _PROGRAM is None:
        _PROGRAM = _build_program()
    return _PROGRAM


def _pack_blocks(x):
    """[4096, W] -> [128, NB*W] with partition p, block b at cols [b*W,(b+1)*W)."""
    W = x.shape[1]
    return np.ascontiguousarray(
        x.reshape(NB, P, W).transpose(1, 0, 2).reshape(P, NB * W)
    )


def _make_in_maps(modal1, modal2, targets):
    x1 = np.asarray(modal1, dtype=np.float32)
    x2 = np.asarray(modal2, dtype=np.float32)
    targets = np.asarray(targets)

    # bf16 hi/lo split (exact to ~2^-17 relative)
    def hilo(x):
        hi = x.astype(NPBF16)
        lo = (x - hi.astype(np.float32)).astype(NPBF16)
        return hi, lo

    x1_hi, x1_lo = hilo(x1)
    x2_hi, x2_lo = hilo(x2)

    oh = (targets[:, None] == np.arange(C)[None, :]).astype(NPBF16)  # [N, C]
    oh_packed = _pack_blocks(oh)

    in_maps = []
    for k in range(NCORES):
        sl = slice(k * DCHUNK, (k + 1) * DCHUNK)

        def pack_modal(hi, lo):
            # [4096, 512] = hi | lo for this core's d-chunk
            hl = np.concatenate([hi[:, sl], lo[:, sl]], axis=1)
            return _pack_blocks(hl)

        in_maps.append(
            {
                "oh": oh_packed,
                "x1": pack_modal(x1_hi, x1_lo),
                "x2": pack_modal(x2_hi, x2_lo),
            }
        )
    return in_maps


def _finish_on_host(sums_list, targets):
    """Recombine hi/lo sums, form class Grams, and do the class-pair loss."""
    P1 = np.zeros((C, C), np.float64)
    P2 = np.zeros((C, C), np.float64)
    P3 = np.zeros((C, C), np.float64)
    for s in sums_list:
        s = np.asarray(s, np.float64)
        R = s[:, 0:256] + s[:, 256:512]      # [class, d-chunk]
        T = s[:, 512:768] + s[:, 768:1024]
        P1 += R @ R.T
        P2 += T @ T.T
        P3 += R @ T.T

    n = np.bincount(targets, minlength=C).astype(np.float64)
    u = 1.0 / np.maximum(n, 1.0)

    S_CC = P1 + P2 + P3 + P3.T  # (R+T)(R+T)^T
    uu = np.outer(u, u)
    A1 = 0.5 * uu * (P1 + P3)    # meanR . ctr
    A2 = 0.5 * uu * (P2 + P3.T)  # meanT . ctr
    nR = u * u * np.diag(P1)
    nT = u * u * np.diag(P2)
    nCtr = 0.25 * u * u * np.diag(S_CC)

    W = np.outer(n, n)
    eye = np.eye(C)
    total = 0.0
    for A, nrm in ((A1, nR), (A2, nT)):
        sq = np.maximum(nrm[:, None] + nCtr[None, :] - 2.0 * A, 1e-12)
        d = np.sqrt(sq)
        dd = np.sqrt(d + 1e-10)
        term = eye * sq + (1.0 - eye) * np.maximum(MARGIN - dd, 0.0) ** 2
        total += (W * term).sum() / (float(N) * float(N))
    return np.asarray(total, dtype=np.float32)


def kernel(modal1_inputs, modal2_inputs, targets):
    nc = _get_program()
    in_maps = _make_in_maps(modal1_inputs, modal2_inputs, targets)
    res = run_bass_kernel_spmd(nc, in_maps, list(range(NCORES)))
    sums_list = [res.results[k]["sums"] for k in range(NCORES)]
    return _finish_on_host(sums_list, np.asarray(targets))
